# revision 17
# baseline (speedup 1.0000x reference)
"""GINEConv + 2-layer MLP + residual + BatchNorm on 8 Trainium2 NeuronCores.

Strategy (graph/data parallel, per sharding hint):
- Partition dst nodes contiguously across 8 cores (6272 nodes/core, core 7
  padded). Each core owns the edges incident to its dst nodes.
- Host preprocessing: per core, group edges by 128-node dst windows; within
  each window split edges into lo-src (< 25088) / hi-src halves (so gather
  indices fit int16), pad each half to a cross-core-uniform multiple of 128
  so the SPMD program is identical on every core. Pad edges use
  edge_attr=-1e30 so relu(x[src]+ea) == 0.
- Device: x replicated in DRAM as two bf16 half-tables. Per window-part,
  batched SWDGE dma_gather calls (<=1024 rows each) fetch x[src] rows in
  bf16, 256B/row; edge_attr streams as bf16 in a partition-major chunked
  layout. msg = relu(xg + ea) in bf16; one-hot S[e, m] = (rel_dst[e] == m)
  via iota/is_equal; aggr^T[f, m] += msg^T S with bf16 TensorE matmuls
  accumulated in fp32 PSUM.
- Per window: h = x + aggr; h2 = x + (relu(h@W1+b1)@W2+b2) in fp32 with
  weights stationary; per-feature partial sums for BN accumulated on the fly.
- BN: one AllReduce of [128, 2] (sum, sumsq), biased variance, then a
  normalize + PE-transpose + store pass.

kernel(**inputs) takes FULL inputs, returns FULL [50000, 128] output.
"""
import numpy as np
import ml_dtypes

import concourse.bass as bass
import concourse.mybir as mybir
import concourse.tile as tile
import concourse.bacc as bacc
import concourse.bass_utils as bass_utils
from concourse.masks import make_identity

P = 128
D = 128
NCORES = 8
BN_EPS = 1e-5
NEG = -1.0e30
HALF = 25088          # split of the node id space for int16 gather indices
MAXIDX = 1024         # max rows per dma_gather call on this runtime

F32 = mybir.dt.float32
BF16 = mybir.dt.bfloat16
I16 = mybir.dt.int16

BF = ml_dtypes.bfloat16


# ----------------------------------------------------------------- host prep
def _prep(x, edge_index, edge_attr):
    """Partition + pad edges; build per-core arrays (identical shapes)."""
    N = x.shape[0]
    npc = ((N + NCORES - 1) // NCORES + P - 1) // P * P     # 6272
    wn = 224 if npc % 224 == 0 else P   # dst-window node count
    nw = npc // wn                      # 28 for the full problem
    src = edge_index[0].astype(np.int64)
    dst = edge_index[1].astype(np.int64)
    core = np.minimum(dst // npc, NCORES - 1)
    ldst = dst - core * npc
    win = ldst // wn
    rel = ldst % wn
    half = (src >= HALF).astype(np.int64)

    # per (core, window, half) counts -> cross-core uniform subtile counts
    counts = np.zeros((NCORES, nw, 2), np.int64)
    np.add.at(counts, (core, win, half), 1)
    pw = np.maximum(1, (counts.max(axis=0) + P - 1) // P)    # [nw, 2]
    nsub = int(pw.sum())
    epad = nsub * P

    # slot space: [all lo parts (by window) | all hi parts (by window)] so
    # each half is one contiguous gather stream of uniform 1024-row calls
    part_order = [(w, 0) for w in range(nw)] + [(w, 1) for w in range(nw)]
    part_sizes = np.array([pw[w, h] * P for (w, h) in part_order])
    part_starts = np.concatenate([[0], part_sizes.cumsum()])[:-1]
    pstart = np.zeros((nw, 2), np.int64)
    for i, (w, h) in enumerate(part_order):
        pstart[w, h] = part_starts[i]

    # order edges by (half, window, core); scatter edge ids into slots
    order = np.lexsort((win, half, core))
    core_o = core[order]
    src_pc = np.full((NCORES, epad), -1, np.int64)
    ptr = 0
    for c in range(NCORES):
        n_c = int((core_o == c).sum())
        ce = order[ptr:ptr + n_c]                 # edge ids, (half, win)-sorted
        wc = np.array([counts[c, w, h] for (w, h) in part_order])
        offs = np.repeat(part_starts, wc)
        inner = np.arange(n_c) - np.repeat(
            np.concatenate([[0], wc.cumsum()])[:-1], wc)
        src_pc[c, offs + inner] = ce
        ptr += n_c

    ea_host = np.empty((NCORES, P, nsub * D), BF)
    idx16 = np.zeros((NCORES, epad), np.int16)
    rels = np.zeros((NCORES, epad), np.float32)
    for c in range(NCORES):
        sel = src_pc[c]
        valid = sel >= 0
        eac = np.full((epad, D), NEG, np.float32)
        eac[valid] = edge_attr[sel[valid]]
        # [slot, f] -> [p, s, f] partition-major chunk layout
        ea_host[c] = eac.astype(BF).reshape(nsub, P, D).transpose(
            1, 0, 2).reshape(P, nsub * D)
        sv = src[sel[valid]]
        idx16[c, valid] = np.where(sv >= HALF, sv - HALF, sv).astype(np.int16)
        rels[c, valid] = rel[sel[valid]].astype(np.float32)
        rels[c, ~valid] = 300.0   # out-of-range -> one-hot col all zero

    # wrapped int16 gather index table: within each 128-slot subtile group,
    # wrapping is per gather call; calls are whole-subtile aligned, and the
    # wrap pattern (j -> partition j%16, col j//16) is position-independent
    # as long as call starts are 16-multiples in col space (128-mult in slots)
    idx_w = np.empty((NCORES, P, epad // 16), np.int16)
    for c in range(NCORES):
        blk = idx16[c].reshape(epad // 16, 16).T     # [16, epad/16]
        idx_w[c] = np.tile(blk, (8, 1))

    rel_pt = rels.reshape(NCORES, nsub, P).transpose(0, 2, 1).copy()

    xt = np.zeros((NCORES, D, npc), np.float32)
    for c in range(NCORES):
        lo = c * npc
        hi = min(N, lo + npc)
        xt[c, :, :hi - lo] = x[lo:hi].T
    xt_g = xt.reshape(NCORES, D, nw, wn).transpose(0, 2, 1, 3).copy()

    # bf16 gather half-tables (replicated to every core); rows padded to a
    # 512B stride — measured 35% faster random gather than packed 256B rows
    NPAD = 2 * HALF                                          # 50176
    xb = np.zeros((NPAD, D), BF)
    xb[:N] = x.astype(BF)
    xlo = np.zeros((HALF, 2 * D), BF)
    xlo[:, :D] = xb[:HALF]
    xhi = np.zeros((HALF, 2 * D), BF)
    xhi[:, :D] = xb[HALF:]

    npad_nodes = np.zeros((NCORES, P), np.float32)
    npad_nodes[NCORES - 1, :] = NCORES * npc - N
    return dict(nw=nw, pw=pw, pstart=pstart, nsub=nsub, epad=epad, npc=npc,
                ea_host=ea_host, idx_w=idx_w, rel_pt=rel_pt, xt_g=xt_g,
                xlo=xlo, xhi=xhi, npad=npad_nodes)


# ------------------------------------------------------------- device program
def build_nc(nw, pw, pstart, nsub, epad, N, repeat=1, ablate=(),
             unroll=1):
    nc = bacc.Bacc("TRN2", target_bir_lowering=False, debug=False,
                   num_devices=NCORES)
    t_xlo = nc.dram_tensor("xlo", [HALF, 2 * D], BF16,
                           kind="ExternalInput").ap()
    t_xhi = nc.dram_tensor("xhi", [HALF, 2 * D], BF16,
                           kind="ExternalInput").ap()
    t_ea = nc.dram_tensor("ea", [P, nsub * D], BF16, kind="ExternalInput").ap()
    t_idx = nc.dram_tensor("idxs", [P, epad // 16], I16,
                           kind="ExternalInput").ap()
    t_rel = nc.dram_tensor("rels", [P, nsub], F32, kind="ExternalInput").ap()
    npc_d = ((N + NCORES - 1) // NCORES + P - 1) // P * P
    WN = npc_d // nw                    # window node count (224 or 128)
    t_xt = nc.dram_tensor("xt", [nw, P, WN], F32, kind="ExternalInput").ap()
    t_w1 = nc.dram_tensor("W1", [D, D], F32, kind="ExternalInput").ap()
    t_w2 = nc.dram_tensor("W2", [D, D], F32, kind="ExternalInput").ap()
    t_b1 = nc.dram_tensor("b1", [D], F32, kind="ExternalInput").ap()
    t_b2 = nc.dram_tensor("b2", [D], F32, kind="ExternalInput").ap()
    t_bnw = nc.dram_tensor("bn_w", [D], F32, kind="ExternalInput").ap()
    t_bnb = nc.dram_tensor("bn_b", [D], F32, kind="ExternalInput").ap()
    t_npad = nc.dram_tensor("npad", [P], F32, kind="ExternalInput").ap()
    t_out = nc.dram_tensor("out", [nw * WN, D], F32,
                           kind="ExternalOutput").ap()

    # two packed gather regions: lo subtiles [0, NL), hi [NL, NL+NH);
    # uniform chunks of MAXIDX/P subtiles (tails smaller)
    CS = MAXIDX // P
    NL = int(pw[:, 0].sum())
    NH = int(pw[:, 1].sum())

    def chunk_of(gsub):
        # -> (half, chunk_base_subtile, chunk_size, k_within_chunk)
        if gsub < NL:
            base = (gsub // CS) * CS
            return 0, base, min(CS, NL - base), gsub - base
        g = gsub - NL
        base = NL + (g // CS) * CS
        return 1, base, min(CS, NL + NH - base), gsub - base

    with tile.TileContext(nc) as tc:
        with (
            tc.tile_pool(name="const", bufs=1) as cpool,
            tc.tile_pool(name="io", bufs=12) as io,
            tc.tile_pool(name="work", bufs=8) as work,
            tc.tile_pool(name="h2p", bufs=nw + 1) as h2p,
            tc.tile_pool(name="psA", bufs=2, space="PSUM") as psA,
            tc.tile_pool(name="psB", bufs=2, space="PSUM") as psB,
            tc.tile_pool(name="psC", bufs=2, space="PSUM") as psC,
            tc.tile_pool(name="psD", bufs=2, space="PSUM") as psD,
            tc.tile_pool(name="dram", bufs=2, space="DRAM") as dram,
        ):
            # ---- constants (outside any repeat loop)
            w1_sb = cpool.tile([P, D], F32)
            nc.sync.dma_start(out=w1_sb[:], in_=t_w1[:])
            w2_sb = cpool.tile([P, D], F32)
            nc.sync.dma_start(out=w2_sb[:], in_=t_w2[:])
            b1_sb = cpool.tile([P, 1], F32)
            nc.sync.dma_start(out=b1_sb[:], in_=t_b1[:, None])
            b2_sb = cpool.tile([P, 1], F32)
            nc.sync.dma_start(out=b2_sb[:], in_=t_b2[:, None])
            bnw_sb = cpool.tile([P, 1], F32)
            nc.sync.dma_start(out=bnw_sb[:], in_=t_bnw[:, None])
            bnb_sb = cpool.tile([P, 1], F32)
            nc.sync.dma_start(out=bnb_sb[:], in_=t_bnb[:, None])
            npad_sb = cpool.tile([P, 1], F32)
            nc.sync.dma_start(out=npad_sb[:], in_=t_npad[:, None])
            idx_sb = cpool.tile([P, epad // 16], I16)
            nc.sync.dma_start(out=idx_sb[:], in_=t_idx[:])
            rel_sb = cpool.tile([P, nsub], F32)
            nc.sync.dma_start(out=rel_sb[:], in_=t_rel[:])
            iota_i = cpool.tile([P, WN], mybir.dt.int32)
            nc.gpsimd.iota(iota_i[:], pattern=[[1, WN]], base=0,
                           channel_multiplier=0)
            ident = cpool.tile([P, P], F32)
            make_identity(nc, ident[:])
            iota_b = cpool.tile([P, WN], BF16)
            nc.vector.tensor_copy(out=iota_b[:], in_=iota_i[:])

            sums = cpool.tile([P, nw], F32)
            sumsq = cpool.tile([P, nw], F32)

            def emit_main():
                # ================= main streaming pass =================
                h2_tiles = []
                chunks = {}

                def ensure_chunk(gsub):
                    h, base, csz, k = chunk_of(gsub)
                    if base not in chunks:
                        t_x_half = t_xlo if h == 0 else t_xhi
                        g = io.tile([P, csz, P], BF16, tag="g")
                        nc.gpsimd.dma_gather(
                            out_ap=g[:], in_ap=t_x_half[:, 0:D],
                            idxs_ap=idx_sb[:, base * 8:(base + csz) * 8],
                            num_idxs=csz * P, num_idxs_reg=csz * P,
                            elem_size=P, elem_step=2 * D)
                        ea_c = io.tile([P, csz * P], BF16, tag="ea")
                        nc.sync.dma_start(
                            out=ea_c[:],
                            in_=t_ea[:, base * P:(base + csz) * P])
                        chunks[base] = (g, ea_c)
                    g, ea_c = chunks[base]
                    return g, ea_c, k

                def finish_window(w, aggr, xt_w):
                    # h = x + aggr  (feat on partitions, nodes on free)
                    hpre = work.tile([P, WN], F32, tag="hpre")
                    nc.vector.tensor_add(out=hpre[:], in0=aggr[:],
                                         in1=xt_w[:])
                    mm1 = psB.tile([P, WN], F32, space="PSUM", tag="mm1")
                    nc.tensor.matmul(out=mm1[:], lhsT=w1_sb[:], rhs=hpre[:],
                                     start=True, stop=True)
                    r1 = work.tile([P, WN], F32, tag="r1")
                    nc.scalar.activation(
                        out=r1[:], in_=mm1[:],
                        func=mybir.ActivationFunctionType.Relu,
                        bias=b1_sb[:, :1])
                    mm2 = psC.tile([P, WN], F32, space="PSUM", tag="mm2")
                    nc.tensor.matmul(out=mm2[:], lhsT=w2_sb[:], rhs=r1[:],
                                     start=True, stop=True)
                    h2_t = h2p.tile([P, WN], F32, tag="h2")
                    nc.vector.scalar_tensor_tensor(
                        out=h2_t[:], in0=mm2[:], scalar=b2_sb[:, :1],
                        in1=xt_w[:], op0=mybir.AluOpType.add,
                        op1=mybir.AluOpType.add, accum_out=sums[:, w:w + 1])
                    sqs = work.tile([P, WN], F32, tag="sqs")
                    nc.scalar.activation(
                        out=sqs[:], in_=h2_t[:],
                        func=mybir.ActivationFunctionType.Square,
                        accum_out=sumsq[:, w:w + 1])
                    h2_tiles.append(h2_t)

                for w in range(nw):
                    aggr_ps = psA.tile([P, WN], F32, space="PSUM",
                                       tag="aggr")
                    xt_w = io.tile([P, WN], F32, tag="xtw")
                    nc.sync.dma_start(out=xt_w[:], in_=t_xt[w, :, :])
                    w_total = int(pw[w, 0] + pw[w, 1])
                    done = 0
                    for h in range(2):
                        psub = int(pstart[w, h]) // P
                        for t in range(int(pw[w, h])):
                            j = psub + t
                            g, ea_c, k = ensure_chunk(j)
                            if "msg" in ablate:
                                msg_t = work.tile([P, D], BF16, tag="msg")
                                nc.scalar.activation(
                                    out=msg_t[:],
                                    in_=ea_c[:, k * P:(k + 1) * P],
                                    func=mybir.ActivationFunctionType.Relu)
                            else:
                                sum_t = work.tile([P, D], BF16, tag="sum")
                                nc.vector.tensor_add(
                                    out=sum_t[:], in0=g[:, k, :],
                                    in1=ea_c[:, k * P:(k + 1) * P])
                                msg_t = work.tile([P, D], BF16, tag="msg")
                                nc.scalar.activation(
                                    out=msg_t[:], in_=sum_t[:],
                                    func=mybir.ActivationFunctionType.Relu)
                            if "S" in ablate:
                                s_t = iota_b
                            else:
                                s_t = work.tile([P, WN], BF16, tag="S")
                                seng = (nc.gpsimd if "gs" in ablate
                                        else nc.vector)
                                seng.tensor_scalar(
                                    out=s_t[:], in0=iota_b[:],
                                    scalar1=rel_sb[:, j:j + 1], scalar2=None,
                                    op0=mybir.AluOpType.is_equal)
                            nc.tensor.matmul(
                                out=aggr_ps[:], lhsT=msg_t[:], rhs=s_t[:],
                                start=bool(done == 0),
                                stop=bool(done == w_total - 1))
                            done += 1
                    finish_window(w, aggr_ps, xt_w)
                return h2_tiles

            def emit_norm(h2_tiles, alpha_ap, beta_ap):
                # normalize + transpose (in <=128-col segments) + store
                segs = [(s0, min(P, WN - s0)) for s0 in range(0, WN, P)]
                for w in range(nw):
                    nrm = work.tile([P, WN], F32, tag="nrm")
                    nc.vector.tensor_scalar(
                        out=nrm[:], in0=h2_tiles[w][:], scalar1=alpha_ap,
                        scalar2=beta_ap, op0=mybir.AluOpType.mult,
                        op1=mybir.AluOpType.add)
                    for (s0, sl) in segs:
                        tps = psD.tile([P, P], F32, space="PSUM", tag="tp")
                        nc.tensor.transpose(out=tps[:sl, :],
                                            in_=nrm[:, s0:s0 + sl],
                                            identity=ident[:])
                        ot = work.tile([P, P], F32, tag="ot")
                        nc.scalar.copy(out=ot[:sl, :], in_=tps[:sl, :])
                        nc.sync.dma_start(
                            out=t_out[w * WN + s0:w * WN + s0 + sl, :],
                            in_=ot[:sl, :])

            if repeat > 1:
                # timing mode: loop main + normalize (dummy scale/shift);
                # excludes only the one-time [128,2] AllReduce/stats chain
                assert repeat % unroll == 0
                with tc.For_i(0, repeat // unroll, 1):
                    for _ in range(unroll):
                        h2_tiles = emit_main()
                        if "norm" not in ablate:
                            emit_norm(h2_tiles, bnw_sb[:, :1],
                                      bnb_sb[:, :1])
            h2_tiles = emit_main()

            if repeat == 1:
                # ================= BN statistics =================
                # pad-node correction: c = W2^T relu(b1) + b2
                rb1 = cpool.tile([P, 1], F32)
                nc.scalar.activation(out=rb1[:], in_=b1_sb[:],
                                     func=mybir.ActivationFunctionType.Relu)
                cps = psB.tile([P, 1], F32, space="PSUM", tag="mm1")
                nc.tensor.matmul(out=cps[:], lhsT=w2_sb[:], rhs=rb1[:],
                                 start=True, stop=True)
                cvec = cpool.tile([P, 1], F32)
                nc.vector.tensor_add(out=cvec[:], in0=cps[:], in1=b2_sb[:])

                part = cpool.tile([P, 2], F32)
                nc.vector.tensor_reduce(out=part[:, 0:1], in_=sums[:],
                                        axis=mybir.AxisListType.X,
                                        op=mybir.AluOpType.add)
                nc.vector.tensor_reduce(out=part[:, 1:2], in_=sumsq[:],
                                        axis=mybir.AxisListType.X,
                                        op=mybir.AluOpType.add)
                corr = cpool.tile([P, 2], F32)
                nc.vector.tensor_mul(out=corr[:, 0:1], in0=npad_sb[:],
                                     in1=cvec[:])
                csq = cpool.tile([P, 1], F32)
                nc.vector.tensor_mul(out=csq[:], in0=cvec[:], in1=cvec[:])
                nc.vector.tensor_mul(out=corr[:, 1:2], in0=npad_sb[:],
                                     in1=csq[:])
                nc.vector.tensor_sub(out=part[:], in0=part[:], in1=corr[:])

                cin = dram.tile([P, 2], F32)
                cout = dram.tile([P, 2], F32)
                nc.sync.dma_start(out=cin[:], in_=part[:])
                nc.gpsimd.collective_compute(
                    "AllReduce", mybir.AluOpType.add,
                    replica_groups=[list(range(NCORES))],
                    ins=[cin.opt()], outs=[cout.opt()])
                stats = cpool.tile([P, 2], F32)
                nc.sync.dma_start(out=stats[:], in_=cout[:])

                inv_n = 1.0 / float(N)
                mean = cpool.tile([P, 1], F32)
                nc.vector.tensor_scalar(out=mean[:], in0=stats[:, 0:1],
                                        scalar1=inv_n, scalar2=None,
                                        op0=mybir.AluOpType.mult)
                msq = cpool.tile([P, 1], F32)
                nc.vector.tensor_scalar(out=msq[:], in0=stats[:, 1:2],
                                        scalar1=inv_n, scalar2=None,
                                        op0=mybir.AluOpType.mult)
                m2 = cpool.tile([P, 1], F32)
                nc.vector.tensor_mul(out=m2[:], in0=mean[:], in1=mean[:])
                var = cpool.tile([P, 1], F32)
                nc.vector.tensor_sub(out=var[:], in0=msq[:], in1=m2[:])
                vare = cpool.tile([P, 1], F32)
                nc.vector.tensor_scalar(out=vare[:], in0=var[:],
                                        scalar1=BN_EPS, scalar2=None,
                                        op0=mybir.AluOpType.add)
                std = cpool.tile([P, 1], F32)
                nc.scalar.activation(out=std[:], in_=vare[:],
                                     func=mybir.ActivationFunctionType.Sqrt)
                inv = cpool.tile([P, 1], F32)
                nc.vector.reciprocal(out=inv[:], in_=std[:])
                alpha = cpool.tile([P, 1], F32)
                nc.vector.tensor_mul(out=alpha[:], in0=inv[:], in1=bnw_sb[:])
                am = cpool.tile([P, 1], F32)
                nc.vector.tensor_mul(out=am[:], in0=mean[:], in1=alpha[:])
                beta = cpool.tile([P, 1], F32)
                nc.vector.tensor_sub(out=beta[:], in0=bnb_sb[:], in1=am[:])

                # ================= normalize + transpose + store ============
                emit_norm(h2_tiles, alpha[:, :1], beta[:, :1])

    nc.compile()
    return nc


# ----------------------------------------------------------------- entrypoint
_CACHE = {}


def kernel(x, edge_index, edge_attr, W1, b1, W2, b2, bn_w, bn_b):
    x = np.asarray(x, dtype=np.float32)
    edge_index = np.asarray(edge_index, dtype=np.int32)
    edge_attr = np.asarray(edge_attr, dtype=np.float32)
    N = x.shape[0]
    pp = _prep(x, edge_index, edge_attr)
    key = (N, pp["nsub"], tuple(pp["pw"].reshape(-1).tolist()))
    if key not in _CACHE:
        _CACHE[key] = build_nc(pp["nw"], pp["pw"], pp["pstart"], pp["nsub"],
                               pp["epad"], N)
    nc = _CACHE[key]

    in_maps = make_in_maps(pp, x, W1, b1, W2, b2, bn_w, bn_b)
    res = bass_utils.run_bass_kernel_spmd(nc, in_maps,
                                          core_ids=list(range(NCORES)))
    npc = pp["npc"]
    out = np.empty((N, D), np.float32)
    for c in range(NCORES):
        lo = c * npc
        hi = min(N, lo + npc)
        out[lo:hi] = res.results[c]["out"][:hi - lo]
    return out


def make_in_maps(pp, x, W1, b1, W2, b2, bn_w, bn_b):
    in_maps = []
    for c in range(NCORES):
        in_maps.append(dict(
            xlo=pp["xlo"], xhi=pp["xhi"], ea=pp["ea_host"][c],
            idxs=pp["idx_w"][c], rels=pp["rel_pt"][c], xt=pp["xt_g"][c],
            W1=np.asarray(W1, np.float32), W2=np.asarray(W2, np.float32),
            b1=np.asarray(b1, np.float32), b2=np.asarray(b2, np.float32),
            bn_w=np.asarray(bn_w, np.float32),
            bn_b=np.asarray(bn_b, np.float32),
            npad=pp["npad"][c],
        ))
    return in_maps


# revision 19
# speedup vs baseline: 1.1980x; 1.1980x over previous
"""GINEConv + 2-layer MLP + residual + BatchNorm on 8 Trainium2 NeuronCores.

Strategy (graph/data parallel, per sharding hint):
- Partition dst nodes contiguously across 8 cores (6272 nodes/core, core 7
  padded). Each core owns the edges incident to its dst nodes.
- Host preprocessing: per core, group edges by 128-node dst windows; within
  each window split edges into lo-src (< 25088) / hi-src halves (so gather
  indices fit int16), pad each half to a cross-core-uniform multiple of 128
  so the SPMD program is identical on every core. Pad edges use
  edge_attr=-1e30 so relu(x[src]+ea) == 0.
- Device: x replicated in DRAM as two bf16 half-tables. Per window-part,
  batched SWDGE dma_gather calls (<=1024 rows each) fetch x[src] rows in
  bf16, 256B/row; edge_attr streams as bf16 in a partition-major chunked
  layout. msg = relu(xg + ea) in bf16; one-hot S[e, m] = (rel_dst[e] == m)
  via iota/is_equal; aggr^T[f, m] += msg^T S with bf16 TensorE matmuls
  accumulated in fp32 PSUM.
- Per window: h = x + aggr; h2 = x + (relu(h@W1+b1)@W2+b2) in fp32 with
  weights stationary; per-feature partial sums for BN accumulated on the fly.
- BN: one AllReduce of [128, 2] (sum, sumsq), biased variance, then a
  normalize + PE-transpose + store pass.

kernel(**inputs) takes FULL inputs, returns FULL [50000, 128] output.
"""
import numpy as np
import ml_dtypes

import concourse.bass as bass
import concourse.mybir as mybir
import concourse.tile as tile
import concourse.bacc as bacc
import concourse.bass_utils as bass_utils
from concourse.masks import make_identity

P = 128
D = 128
NCORES = 8
BN_EPS = 1e-5
NEG = -1.0e30
HALF = 25088          # split of the node id space for int16 gather indices
MAXIDX = 1024         # max rows per dma_gather call on this runtime

F32 = mybir.dt.float32
BF16 = mybir.dt.bfloat16
I16 = mybir.dt.int16

BF = ml_dtypes.bfloat16


# ----------------------------------------------------------------- host prep
def _prep(x, edge_index, edge_attr, stride512=False):
    """Partition + pad edges; build per-core arrays (identical shapes)."""
    N = x.shape[0]
    npc = ((N + NCORES - 1) // NCORES + P - 1) // P * P     # 6272
    wn = 224 if npc % 224 == 0 else P   # dst-window node count
    nw = npc // wn                      # 28 for the full problem
    src = edge_index[0].astype(np.int64)
    dst = edge_index[1].astype(np.int64)
    core = np.minimum(dst // npc, NCORES - 1)
    ldst = dst - core * npc
    win = ldst // wn
    rel = ldst % wn
    half = (src >= HALF).astype(np.int64)

    # per (core, window, half) counts -> cross-core uniform subtile counts
    counts = np.zeros((NCORES, nw, 2), np.int64)
    np.add.at(counts, (core, win, half), 1)
    pw = np.maximum(1, (counts.max(axis=0) + P - 1) // P)    # [nw, 2]
    nsub = int(pw.sum())
    epad = nsub * P

    # slot space: [all lo parts (by window) | all hi parts (by window)] so
    # each half is one contiguous gather stream of uniform 1024-row calls
    part_order = [(w, 0) for w in range(nw)] + [(w, 1) for w in range(nw)]
    part_sizes = np.array([pw[w, h] * P for (w, h) in part_order])
    part_starts = np.concatenate([[0], part_sizes.cumsum()])[:-1]
    pstart = np.zeros((nw, 2), np.int64)
    for i, (w, h) in enumerate(part_order):
        pstart[w, h] = part_starts[i]

    # order edges by (half, window, core); scatter edge ids into slots
    order = np.lexsort((win, half, core))
    core_o = core[order]
    src_pc = np.full((NCORES, epad), -1, np.int64)
    ptr = 0
    for c in range(NCORES):
        n_c = int((core_o == c).sum())
        ce = order[ptr:ptr + n_c]                 # edge ids, (half, win)-sorted
        wc = np.array([counts[c, w, h] for (w, h) in part_order])
        offs = np.repeat(part_starts, wc)
        inner = np.arange(n_c) - np.repeat(
            np.concatenate([[0], wc.cumsum()])[:-1], wc)
        src_pc[c, offs + inner] = ce
        ptr += n_c

    ea_host = np.empty((NCORES, P, nsub * D), BF)
    idx16 = np.zeros((NCORES, epad), np.int16)
    rels = np.zeros((NCORES, epad), np.float32)
    for c in range(NCORES):
        sel = src_pc[c]
        valid = sel >= 0
        eac = np.full((epad, D), NEG, np.float32)
        eac[valid] = edge_attr[sel[valid]]
        # [slot, f] -> [p, s, f] partition-major chunk layout
        ea_host[c] = eac.astype(BF).reshape(nsub, P, D).transpose(
            1, 0, 2).reshape(P, nsub * D)
        sv = src[sel[valid]]
        idx16[c, valid] = np.where(sv >= HALF, sv - HALF, sv).astype(np.int16)
        rels[c, valid] = rel[sel[valid]].astype(np.float32)
        rels[c, ~valid] = 300.0   # out-of-range -> one-hot col all zero

    # wrapped int16 gather index table: within each 128-slot subtile group,
    # wrapping is per gather call; calls are whole-subtile aligned, and the
    # wrap pattern (j -> partition j%16, col j//16) is position-independent
    # as long as call starts are 16-multiples in col space (128-mult in slots)
    idx_w = np.empty((NCORES, P, epad // 16), np.int16)
    for c in range(NCORES):
        blk = idx16[c].reshape(epad // 16, 16).T     # [16, epad/16]
        idx_w[c] = np.tile(blk, (8, 1))

    rel_pt = rels.reshape(NCORES, nsub, P).transpose(0, 2, 1).copy()

    xt = np.zeros((NCORES, D, npc), np.float32)
    for c in range(NCORES):
        lo = c * npc
        hi = min(N, lo + npc)
        xt[c, :, :hi - lo] = x[lo:hi].T
    xt_g = xt.reshape(NCORES, D, nw, wn).transpose(0, 2, 1, 3).copy()

    # bf16 gather half-tables (replicated to every core)
    NPAD = 2 * HALF                                          # 50176
    xb = np.zeros((NPAD, D), BF)
    xb[:N] = x.astype(BF)
    if stride512:
        xlo = np.zeros((HALF, 2 * D), BF)
        xlo[:, :D] = xb[:HALF]
        xhi = np.zeros((HALF, 2 * D), BF)
        xhi[:, :D] = xb[HALF:]
    else:
        xlo, xhi = xb[:HALF].copy(), xb[HALF:].copy()

    npad_nodes = np.zeros((NCORES, P), np.float32)
    npad_nodes[NCORES - 1, :] = NCORES * npc - N
    return dict(nw=nw, pw=pw, pstart=pstart, nsub=nsub, epad=epad, npc=npc,
                ea_host=ea_host, idx_w=idx_w, rel_pt=rel_pt, xt_g=xt_g,
                xlo=xlo, xhi=xhi, npad=npad_nodes)


# ------------------------------------------------------------- device program
def build_nc(nw, pw, pstart, nsub, epad, N, repeat=1, ablate=(),
             unroll=1, stride512=False):
    nc = bacc.Bacc("TRN2", target_bir_lowering=False, debug=False,
                   num_devices=NCORES)
    XW = 2 * D if stride512 else D
    t_xlo = nc.dram_tensor("xlo", [HALF, XW], BF16, kind="ExternalInput").ap()
    t_xhi = nc.dram_tensor("xhi", [HALF, XW], BF16, kind="ExternalInput").ap()
    t_ea = nc.dram_tensor("ea", [P, nsub * D], BF16, kind="ExternalInput").ap()
    t_idx = nc.dram_tensor("idxs", [P, epad // 16], I16,
                           kind="ExternalInput").ap()
    t_rel = nc.dram_tensor("rels", [P, nsub], F32, kind="ExternalInput").ap()
    npc_d = ((N + NCORES - 1) // NCORES + P - 1) // P * P
    WN = npc_d // nw                    # window node count (224 or 128)
    t_xt = nc.dram_tensor("xt", [nw, P, WN], F32, kind="ExternalInput").ap()
    t_w1 = nc.dram_tensor("W1", [D, D], F32, kind="ExternalInput").ap()
    t_w2 = nc.dram_tensor("W2", [D, D], F32, kind="ExternalInput").ap()
    t_b1 = nc.dram_tensor("b1", [D], F32, kind="ExternalInput").ap()
    t_b2 = nc.dram_tensor("b2", [D], F32, kind="ExternalInput").ap()
    t_bnw = nc.dram_tensor("bn_w", [D], F32, kind="ExternalInput").ap()
    t_bnb = nc.dram_tensor("bn_b", [D], F32, kind="ExternalInput").ap()
    t_npad = nc.dram_tensor("npad", [P], F32, kind="ExternalInput").ap()
    t_out = nc.dram_tensor("out", [nw * WN, D], F32,
                           kind="ExternalOutput").ap()

    # two packed gather regions: lo subtiles [0, NL), hi [NL, NL+NH);
    # uniform chunks of MAXIDX/P subtiles (tails smaller)
    CS = MAXIDX // P
    NL = int(pw[:, 0].sum())
    NH = int(pw[:, 1].sum())

    def chunk_of(gsub):
        # -> (half, chunk_base_subtile, chunk_size, k_within_chunk)
        if gsub < NL:
            base = (gsub // CS) * CS
            return 0, base, min(CS, NL - base), gsub - base
        g = gsub - NL
        base = NL + (g // CS) * CS
        return 1, base, min(CS, NL + NH - base), gsub - base

    with tile.TileContext(nc) as tc:
        with (
            tc.tile_pool(name="const", bufs=1) as cpool,
            tc.tile_pool(name="io", bufs=12) as io,
            tc.tile_pool(name="work", bufs=8) as work,
            tc.tile_pool(name="h2p", bufs=nw + 1) as h2p,
            tc.tile_pool(name="psA", bufs=2, space="PSUM") as psA,
            tc.tile_pool(name="psB", bufs=2, space="PSUM") as psB,
            tc.tile_pool(name="psC", bufs=2, space="PSUM") as psC,
            tc.tile_pool(name="psD", bufs=2, space="PSUM") as psD,
            tc.tile_pool(name="dram", bufs=2, space="DRAM") as dram,
        ):
            # ---- constants (outside any repeat loop)
            w1_sb = cpool.tile([P, D], F32)
            nc.sync.dma_start(out=w1_sb[:], in_=t_w1[:])
            w2_sb = cpool.tile([P, D], F32)
            nc.sync.dma_start(out=w2_sb[:], in_=t_w2[:])
            b1_sb = cpool.tile([P, 1], F32)
            nc.sync.dma_start(out=b1_sb[:], in_=t_b1[:, None])
            b2_sb = cpool.tile([P, 1], F32)
            nc.sync.dma_start(out=b2_sb[:], in_=t_b2[:, None])
            bnw_sb = cpool.tile([P, 1], F32)
            nc.sync.dma_start(out=bnw_sb[:], in_=t_bnw[:, None])
            bnb_sb = cpool.tile([P, 1], F32)
            nc.sync.dma_start(out=bnb_sb[:], in_=t_bnb[:, None])
            npad_sb = cpool.tile([P, 1], F32)
            nc.sync.dma_start(out=npad_sb[:], in_=t_npad[:, None])
            idx_sb = cpool.tile([P, epad // 16], I16)
            nc.sync.dma_start(out=idx_sb[:], in_=t_idx[:])
            rel_sb = cpool.tile([P, nsub], F32)
            nc.sync.dma_start(out=rel_sb[:], in_=t_rel[:])
            iota_i = cpool.tile([P, WN], mybir.dt.int32)
            nc.gpsimd.iota(iota_i[:], pattern=[[1, WN]], base=0,
                           channel_multiplier=0)
            ident = cpool.tile([P, P], F32)
            make_identity(nc, ident[:])
            iota_b = cpool.tile([P, WN], BF16)
            nc.vector.tensor_copy(out=iota_b[:], in_=iota_i[:])

            sums = cpool.tile([P, nw], F32)
            sumsq = cpool.tile([P, nw], F32)

            def emit_main():
                # ================= main streaming pass =================
                h2_tiles = []
                chunks = {}

                def ensure_chunk(gsub):
                    h, base, csz, k = chunk_of(gsub)
                    if base not in chunks:
                        t_x_half = t_xlo if h == 0 else t_xhi
                        g = io.tile([P, csz, P], BF16, tag="g")
                        nc.gpsimd.dma_gather(
                            out_ap=g[:], in_ap=t_x_half[:, 0:D],
                            idxs_ap=idx_sb[:, base * 8:(base + csz) * 8],
                            num_idxs=csz * P, num_idxs_reg=csz * P,
                            elem_size=P, elem_step=XW)
                        ea_c = io.tile([P, csz * P], BF16, tag="ea")
                        nc.sync.dma_start(
                            out=ea_c[:],
                            in_=t_ea[:, base * P:(base + csz) * P])
                        chunks[base] = (g, ea_c)
                    g, ea_c = chunks[base]
                    return g, ea_c, k

                def finish_window(w, aggr, xt_w):
                    # h = x + aggr  (feat on partitions, nodes on free)
                    hpre = work.tile([P, WN], F32, tag="hpre")
                    nc.vector.tensor_add(out=hpre[:], in0=aggr[:],
                                         in1=xt_w[:])
                    mm1 = psB.tile([P, WN], F32, space="PSUM", tag="mm1")
                    nc.tensor.matmul(out=mm1[:], lhsT=w1_sb[:], rhs=hpre[:],
                                     start=True, stop=True)
                    r1 = work.tile([P, WN], F32, tag="r1")
                    nc.scalar.activation(
                        out=r1[:], in_=mm1[:],
                        func=mybir.ActivationFunctionType.Relu,
                        bias=b1_sb[:, :1])
                    mm2 = psC.tile([P, WN], F32, space="PSUM", tag="mm2")
                    nc.tensor.matmul(out=mm2[:], lhsT=w2_sb[:], rhs=r1[:],
                                     start=True, stop=True)
                    h2_t = h2p.tile([P, WN], F32, tag="h2")
                    nc.vector.scalar_tensor_tensor(
                        out=h2_t[:], in0=mm2[:], scalar=b2_sb[:, :1],
                        in1=xt_w[:], op0=mybir.AluOpType.add,
                        op1=mybir.AluOpType.add, accum_out=sums[:, w:w + 1])
                    sqs = work.tile([P, WN], F32, tag="sqs")
                    nc.scalar.activation(
                        out=sqs[:], in_=h2_t[:],
                        func=mybir.ActivationFunctionType.Square,
                        accum_out=sumsq[:, w:w + 1])
                    h2_tiles.append(h2_t)

                for w in range(nw):
                    aggr_ps = psA.tile([P, WN], F32, space="PSUM",
                                       tag="aggr")
                    xt_w = io.tile([P, WN], F32, tag="xtw")
                    nc.sync.dma_start(out=xt_w[:], in_=t_xt[w, :, :])
                    w_total = int(pw[w, 0] + pw[w, 1])
                    done = 0
                    for h in range(2):
                        psub = int(pstart[w, h]) // P
                        for t in range(int(pw[w, h])):
                            j = psub + t
                            g, ea_c, k = ensure_chunk(j)
                            if "msg" in ablate:
                                msg_t = work.tile([P, D], BF16, tag="msg")
                                nc.scalar.activation(
                                    out=msg_t[:],
                                    in_=ea_c[:, k * P:(k + 1) * P],
                                    func=mybir.ActivationFunctionType.Relu)
                            else:
                                sum_t = work.tile([P, D], BF16, tag="sum")
                                nc.vector.tensor_add(
                                    out=sum_t[:], in0=g[:, k, :],
                                    in1=ea_c[:, k * P:(k + 1) * P])
                                msg_t = work.tile([P, D], BF16, tag="msg")
                                nc.scalar.activation(
                                    out=msg_t[:], in_=sum_t[:],
                                    func=mybir.ActivationFunctionType.Relu)
                            if "S" in ablate:
                                s_t = iota_b
                            else:
                                s_t = work.tile([P, WN], BF16, tag="S")
                                seng = (nc.gpsimd if "gs" in ablate
                                        else nc.vector)
                                seng.tensor_scalar(
                                    out=s_t[:], in0=iota_b[:],
                                    scalar1=rel_sb[:, j:j + 1], scalar2=None,
                                    op0=mybir.AluOpType.is_equal)
                            nc.tensor.matmul(
                                out=aggr_ps[:], lhsT=msg_t[:], rhs=s_t[:],
                                start=bool(done == 0),
                                stop=bool(done == w_total - 1))
                            done += 1
                    finish_window(w, aggr_ps, xt_w)
                return h2_tiles

            def emit_norm(h2_tiles, alpha_ap, beta_ap):
                # normalize + transpose (in <=128-col segments) + store
                segs = [(s0, min(P, WN - s0)) for s0 in range(0, WN, P)]
                for w in range(nw):
                    nrm = work.tile([P, WN], F32, tag="nrm")
                    nc.vector.tensor_scalar(
                        out=nrm[:], in0=h2_tiles[w][:], scalar1=alpha_ap,
                        scalar2=beta_ap, op0=mybir.AluOpType.mult,
                        op1=mybir.AluOpType.add)
                    for (s0, sl) in segs:
                        tps = psD.tile([P, P], F32, space="PSUM", tag="tp")
                        nc.tensor.transpose(out=tps[:sl, :],
                                            in_=nrm[:, s0:s0 + sl],
                                            identity=ident[:])
                        ot = work.tile([P, P], F32, tag="ot")
                        nc.scalar.copy(out=ot[:sl, :], in_=tps[:sl, :])
                        nc.sync.dma_start(
                            out=t_out[w * WN + s0:w * WN + s0 + sl, :],
                            in_=ot[:sl, :])

            if repeat > 1:
                # timing mode: loop main + normalize (dummy scale/shift);
                # excludes only the one-time [128,2] AllReduce/stats chain
                assert repeat % unroll == 0
                with tc.For_i(0, repeat // unroll, 1):
                    for _ in range(unroll):
                        h2_tiles = emit_main()
                        if "norm" not in ablate:
                            emit_norm(h2_tiles, bnw_sb[:, :1],
                                      bnb_sb[:, :1])
            h2_tiles = emit_main()

            if repeat == 1:
                # ================= BN statistics =================
                # pad-node correction: c = W2^T relu(b1) + b2
                rb1 = cpool.tile([P, 1], F32)
                nc.scalar.activation(out=rb1[:], in_=b1_sb[:],
                                     func=mybir.ActivationFunctionType.Relu)
                cps = psB.tile([P, 1], F32, space="PSUM", tag="mm1")
                nc.tensor.matmul(out=cps[:], lhsT=w2_sb[:], rhs=rb1[:],
                                 start=True, stop=True)
                cvec = cpool.tile([P, 1], F32)
                nc.vector.tensor_add(out=cvec[:], in0=cps[:], in1=b2_sb[:])

                part = cpool.tile([P, 2], F32)
                nc.vector.tensor_reduce(out=part[:, 0:1], in_=sums[:],
                                        axis=mybir.AxisListType.X,
                                        op=mybir.AluOpType.add)
                nc.vector.tensor_reduce(out=part[:, 1:2], in_=sumsq[:],
                                        axis=mybir.AxisListType.X,
                                        op=mybir.AluOpType.add)
                corr = cpool.tile([P, 2], F32)
                nc.vector.tensor_mul(out=corr[:, 0:1], in0=npad_sb[:],
                                     in1=cvec[:])
                csq = cpool.tile([P, 1], F32)
                nc.vector.tensor_mul(out=csq[:], in0=cvec[:], in1=cvec[:])
                nc.vector.tensor_mul(out=corr[:, 1:2], in0=npad_sb[:],
                                     in1=csq[:])
                nc.vector.tensor_sub(out=part[:], in0=part[:], in1=corr[:])

                cin = dram.tile([P, 2], F32)
                cout = dram.tile([P, 2], F32)
                nc.sync.dma_start(out=cin[:], in_=part[:])
                nc.gpsimd.collective_compute(
                    "AllReduce", mybir.AluOpType.add,
                    replica_groups=[list(range(NCORES))],
                    ins=[cin.opt()], outs=[cout.opt()])
                stats = cpool.tile([P, 2], F32)
                nc.sync.dma_start(out=stats[:], in_=cout[:])

                inv_n = 1.0 / float(N)
                mean = cpool.tile([P, 1], F32)
                nc.vector.tensor_scalar(out=mean[:], in0=stats[:, 0:1],
                                        scalar1=inv_n, scalar2=None,
                                        op0=mybir.AluOpType.mult)
                msq = cpool.tile([P, 1], F32)
                nc.vector.tensor_scalar(out=msq[:], in0=stats[:, 1:2],
                                        scalar1=inv_n, scalar2=None,
                                        op0=mybir.AluOpType.mult)
                m2 = cpool.tile([P, 1], F32)
                nc.vector.tensor_mul(out=m2[:], in0=mean[:], in1=mean[:])
                var = cpool.tile([P, 1], F32)
                nc.vector.tensor_sub(out=var[:], in0=msq[:], in1=m2[:])
                vare = cpool.tile([P, 1], F32)
                nc.vector.tensor_scalar(out=vare[:], in0=var[:],
                                        scalar1=BN_EPS, scalar2=None,
                                        op0=mybir.AluOpType.add)
                std = cpool.tile([P, 1], F32)
                nc.scalar.activation(out=std[:], in_=vare[:],
                                     func=mybir.ActivationFunctionType.Sqrt)
                inv = cpool.tile([P, 1], F32)
                nc.vector.reciprocal(out=inv[:], in_=std[:])
                alpha = cpool.tile([P, 1], F32)
                nc.vector.tensor_mul(out=alpha[:], in0=inv[:], in1=bnw_sb[:])
                am = cpool.tile([P, 1], F32)
                nc.vector.tensor_mul(out=am[:], in0=mean[:], in1=alpha[:])
                beta = cpool.tile([P, 1], F32)
                nc.vector.tensor_sub(out=beta[:], in0=bnb_sb[:], in1=am[:])

                # ================= normalize + transpose + store ============
                emit_norm(h2_tiles, alpha[:, :1], beta[:, :1])

    nc.compile()
    return nc


# ----------------------------------------------------------------- entrypoint
_CACHE = {}


def kernel(x, edge_index, edge_attr, W1, b1, W2, b2, bn_w, bn_b):
    x = np.asarray(x, dtype=np.float32)
    edge_index = np.asarray(edge_index, dtype=np.int32)
    edge_attr = np.asarray(edge_attr, dtype=np.float32)
    N = x.shape[0]
    pp = _prep(x, edge_index, edge_attr)
    key = (N, pp["nsub"], tuple(pp["pw"].reshape(-1).tolist()))
    if key not in _CACHE:
        _CACHE[key] = build_nc(pp["nw"], pp["pw"], pp["pstart"], pp["nsub"],
                               pp["epad"], N)
    nc = _CACHE[key]

    in_maps = make_in_maps(pp, x, W1, b1, W2, b2, bn_w, bn_b)
    res = bass_utils.run_bass_kernel_spmd(nc, in_maps,
                                          core_ids=list(range(NCORES)))
    npc = pp["npc"]
    out = np.empty((N, D), np.float32)
    for c in range(NCORES):
        lo = c * npc
        hi = min(N, lo + npc)
        out[lo:hi] = res.results[c]["out"][:hi - lo]
    return out


def make_in_maps(pp, x, W1, b1, W2, b2, bn_w, bn_b):
    in_maps = []
    for c in range(NCORES):
        in_maps.append(dict(
            xlo=pp["xlo"], xhi=pp["xhi"], ea=pp["ea_host"][c],
            idxs=pp["idx_w"][c], rels=pp["rel_pt"][c], xt=pp["xt_g"][c],
            W1=np.asarray(W1, np.float32), W2=np.asarray(W2, np.float32),
            b1=np.asarray(b1, np.float32), b2=np.asarray(b2, np.float32),
            bn_w=np.asarray(bn_w, np.float32),
            bn_b=np.asarray(bn_b, np.float32),
            npad=pp["npad"][c],
        ))
    return in_maps
